# revision 7
# baseline (speedup 1.0000x reference)
"""Fused attention kernel for Trainium2, 8 NeuronCores.

Problem: B=4, T=2048, C=1024, nh=16, hs=64, fused QKV (chunk order k,q,v),
softmax attention, then (faithful reference bug) reshape (B,nh,T,hs)->(B,T,C)
directly before the output projection.

Key structural fact: with the buggy reshape, head h's attention output
occupies exactly rows [h*128, (h+1)*128) of the reshaped (T, C) matrix
(row tau = h*128 + t//16, col = (t%16)*64 + d). So everything after the
QKV projection is fully independent per (batch, head) pair; the output
projection needs no cross-head reduction.

Sharding: 8 cores = 4 batches x 2 head-groups (8 heads each). Each core
computes its batch's QKV slice and its 8 heads end-to-end. No collectives.
"""

import sys

import numpy as np

sys.path.insert(0, "/opt/trn_rl_repo")

import ml_dtypes  # noqa: E402

B, T, C = 4, 2048, 1024
NH, HS = 16, 64
NCORES = 8
HPC = 8  # heads per core

_CACHE = {}


def _build():
    from contextlib import ExitStack

    import concourse.bass as bass  # noqa: F401
    import concourse.mybir as mybir
    from concourse import bacc, tile

    F32 = mybir.dt.float32
    F32R = mybir.dt.float32r
    BF16 = mybir.dt.bfloat16
    ADD = mybir.AluOpType.add
    MULT = mybir.AluOpType.mult
    EXP = mybir.ActivationFunctionType.Exp

    nc = bacc.Bacc()
    xT = nc.dram_tensor("xT", [128, 8, 2048], F32R, kind="ExternalInput")
    wqkv = nc.dram_tensor("wqkv", [128, 8, 1536], F32R, kind="ExternalInput")
    bqk = nc.dram_tensor("bqk", [128, 8], F32, kind="ExternalInput")
    bv = nc.dram_tensor("bv", [128, 512], F32, kind="ExternalInput")
    wp = nc.dram_tensor("wp", [64, 16, 1024], BF16, kind="ExternalInput")
    pb = nc.dram_tensor("pb", [128, 1024], F32, kind="ExternalInput")
    vones = nc.dram_tensor("vones", [128, 16, 8], BF16, kind="ExternalInput")
    ones1 = nc.dram_tensor("ones1", [1, 64], F32R, kind="ExternalInput")
    y = nc.dram_tensor("y", [128, 8, 1024], F32, kind="ExternalOutput")

    with tile.TileContext(nc) as tc, ExitStack() as ctx:
        persist = ctx.enter_context(tc.tile_pool(name="persist", bufs=1))

        scratch = persist.tile([128, 4], F32, tag="scratch")
        bqk_sb = persist.tile([128, 8], F32, tag="bqk")
        nc.sync.dma_start(bqk_sb, bqk[:])
        nc.vector.tensor_copy(scratch[:, 0:1], bqk_sb[:, 0:1])
        pb_sb = persist.tile([128, 1024], F32, tag="pb")
        nc.sync.dma_start(pb_sb, pb[:])
        nc.vector.tensor_copy(scratch[:, 1:2], pb_sb[:, 0:1])
        ones1_sb = persist.tile([1, 64], F32R, tag="ones1")
        nc.sync.dma_start(ones1_sb, ones1[:])
        vbuf = persist.tile([128, 16, HPC, 65], BF16, tag="vbuf")
        nc.sync.dma_start(vbuf[:, :, :, 64], vones[:])
        nc.vector.tensor_copy(scratch[:, 2:3], vbuf[:, 0, 0, 64:65])
        # K^T rows in tiles 0-3 (d on partitions, t free), Q^T rows in 4-7
        qk = [persist.tile([128, 2048], F32R, tag=f"qk{mt}", name=f"qk{mt}")
              for mt in range(8)]

        # ---------------- QKV phase ----------------
        with tc.tile_pool(name="qkvsb", bufs=1) as qsb, \
             tc.tile_pool(name="wstream", bufs=2) as wsp, \
             tc.tile_pool(name="qkvps", bufs=2, space="PSUM") as qps:
            bv_sb = qsb.tile([128, 512], F32, tag="bv")
            nc.sync.dma_start(bv_sb, bv[:])
            nc.vector.tensor_copy(scratch[:, 3:4], bv_sb[:, 0:1])
            xts = []
            for ct in range(8):
                t = qsb.tile([128, 2048], F32R, tag=f"xt{ct}")
                nc.sync.dma_start(t, xT[:, ct, :])
                xts.append(t)

            for mt in range(8):
                wt = wsp.tile([128, 8, 128], F32R, tag="wt")
                nc.sync.dma_start(wt, wqkv[:, :, mt * 128:(mt + 1) * 128])
                for icx in range(4):
                    ps = qps.tile([128, 512], F32, tag="qkvps")
                    for ct in range(8):
                        nc.tensor.matmul(
                            ps, wt[:, ct, :], xts[ct][:, icx * 512:(icx + 1) * 512],
                            start=(ct == 0), stop=(ct == 7))
                    nc.vector.tensor_tensor(
                        qk[mt][:, icx * 512:(icx + 1) * 512], ps,
                        bqk_sb[:, mt:mt + 1].to_broadcast((128, 512)), ADD)

            wv_sb = qsb.tile([128, 8, 512], F32R, tag="wv")
            nc.sync.dma_start(wv_sb, wqkv[:, :, 1024:1536])
            for tt in range(16):
                ps = qps.tile([128, 512], F32, tag="qkvps")
                for ct in range(8):
                    nc.tensor.matmul(
                        ps, xts[ct][:, tt * 128:(tt + 1) * 128], wv_sb[:, ct, :],
                        start=(ct == 0), stop=(ct == 7))
                nc.vector.tensor_tensor(
                    vbuf[:, tt, :, 0:64],
                    ps.rearrange("p (h d) -> p h d", d=64),
                    bv_sb.rearrange("p (h d) -> p h d", d=64), ADD)

        # ---------------- attention + projection ----------------
        with tc.tile_pool(name="attnsb", bufs=1) as asb, \
             tc.tile_pool(name="utp", bufs=3) as utp, \
             tc.tile_pool(name="otp", bufs=2) as otp, \
             tc.tile_pool(name="nrm", bufs=2) as nrm, \
             tc.tile_pool(name="ysb", bufs=2) as yps, \
             tc.tile_pool(name="spool", bufs=2, space="PSUM") as spx, \
             tc.tile_pool(name="opool", bufs=1, space="PSUM") as opx, \
             tc.tile_pool(name="ypool", bufs=1, space="PSUM") as ypx:
            wp_sb = asb.tile([64, 16, 1024], BF16, tag="wp")
            nc.sync.dma_start(wp_sb, wp[:])
            for h in range(HPC):
                mt = h // 2
                pr = (h % 2) * 64
                kh = qk[mt][pr:pr + 64, :]
                qh = qk[4 + mt][pr:pr + 64, :]
                ot = otp.tile([64, 2048], BF16, tag="ot")
                for ih in range(2):
                    opsum = opx.tile([65, 1024], F32, tag="op")
                    for j in range(16):
                        sp = spx.tile([128, 1024], F32, tag="sp")
                        for q2 in range(2):
                            nc.tensor.matmul(
                                sp[:, q2 * 512:(q2 + 1) * 512],
                                kh[:, j * 128:(j + 1) * 128],
                                qh[:, ih * 1024 + q2 * 512: ih * 1024 + (q2 + 1) * 512],
                                start=True, stop=True)
                        ut = utp.tile([128, 1024], BF16, tag="ut")
                        nc.scalar.activation(ut, sp, EXP, scale=0.125)
                        for q2 in range(2):
                            nc.tensor.matmul(
                                opsum[:, q2 * 512:(q2 + 1) * 512],
                                vbuf[:, j, h, :],
                                ut[:, q2 * 512:(q2 + 1) * 512],
                                start=(j == 0), stop=(j == 15))
                    # normalize: row 64 of opsum is the softmax denominator
                    rs = nrm.tile([1, 1024], F32R, tag="rs")
                    nc.vector.tensor_copy(rs, opsum[64:65, :])
                    bc = spx.tile([64, 1024], F32, tag="sp")
                    for q2 in range(2):
                        nc.tensor.matmul(
                            bc[:, q2 * 512:(q2 + 1) * 512], ones1_sb,
                            rs[:, q2 * 512:(q2 + 1) * 512], start=True, stop=True)
                    rcp = nrm.tile([64, 1024], F32, tag="rcp")
                    nc.vector.reciprocal(rcp, bc)
                    for q2 in range(2):
                        nc.vector.tensor_tensor(
                            ot[:, ih * 1024 + q2 * 512: ih * 1024 + (q2 + 1) * 512],
                            opsum[0:64, q2 * 512:(q2 + 1) * 512],
                            rcp[:, q2 * 512:(q2 + 1) * 512], MULT)
                # projection: Y[tau, o] = sum_u sum_d OT[d, 16*tau+u] * wpT[u*64+d, o]
                yp = ypx.tile([128, 1024], F32, tag="yp")
                otr = ot.rearrange("d (t u) -> d u t", u=16)
                for u in range(16):
                    for q2 in range(2):
                        nc.tensor.matmul(
                            yp[:, q2 * 512:(q2 + 1) * 512], otr[:, u, :],
                            wp_sb[:, u, q2 * 512:(q2 + 1) * 512],
                            start=(u == 0), stop=(u == 15))
                ysb = yps.tile([128, 1024], F32, tag="ysb")
                nc.vector.tensor_tensor(ysb, yp, pb_sb, ADD)
                nc.sync.dma_start(y[:, h, :], ysb)
    nc.compile()
    return nc


def _in_maps(x, w_weight, w_bias, proj_weight, proj_bias):
    x = np.ascontiguousarray(x, np.float32)
    w_weight = np.ascontiguousarray(w_weight, np.float32)
    w_bias = np.ascontiguousarray(w_bias, np.float32)
    proj_weight = np.ascontiguousarray(proj_weight, np.float32)
    proj_bias = np.ascontiguousarray(proj_bias, np.float32)

    wpT = np.ascontiguousarray(
        proj_weight.T.reshape(16, 64, 1024).transpose(1, 0, 2).astype(ml_dtypes.bfloat16))
    pbr = np.ascontiguousarray(np.tile(proj_bias[None], (128, 1)))
    vones = np.ones((128, 16, HPC), dtype=ml_dtypes.bfloat16)
    ones1 = np.ones((1, 64), np.float32)

    maps = []
    for c in range(NCORES):
        b = c // 2
        h0 = (c % 2) * HPC
        xTc = np.ascontiguousarray(
            x[b].T.reshape(8, 128, 2048).transpose(1, 0, 2))
        wk = w_weight[h0 * 64: h0 * 64 + 512]
        wq = w_weight[1024 + h0 * 64: 1024 + h0 * 64 + 512]
        wv = w_weight[2048 + h0 * 64: 2048 + h0 * 64 + 512]
        wqkvT = np.concatenate([wk.T, wq.T, wv.T], axis=1)  # [1024, 1536]
        wqkvT = np.ascontiguousarray(
            wqkvT.reshape(8, 128, 1536).transpose(1, 0, 2))
        bk = w_bias[h0 * 64: h0 * 64 + 512]
        bq = w_bias[1024 + h0 * 64: 1024 + h0 * 64 + 512]
        bvc = w_bias[2048 + h0 * 64: 2048 + h0 * 64 + 512]
        bqkc = np.ascontiguousarray(
            np.concatenate([bk.reshape(4, 128).T, bq.reshape(4, 128).T], axis=1))
        bvr = np.ascontiguousarray(np.tile(bvc[None], (128, 1)))
        maps.append({
            "xT": xTc, "wqkv": wqkvT, "bqk": bqkc, "bv": bvr,
            "wp": wpT, "pb": pbr, "vones": vones, "ones1": ones1,
        })
    return maps


def _install_ntff_hook():
    """Register the axon NTFF profiling hook (missing antenv.axon_hooks shim)."""
    import contextlib
    import ctypes
    import types

    if "antenv.axon_hooks" in sys.modules:
        return
    import antenv
    so_path = "/opt/axon/libaxon_pjrt.so"
    try:
        lib = ctypes.CDLL(so_path)
    except OSError:
        return
    if not hasattr(lib, "axon_start_nrt_profile"):
        return
    lib.axon_start_nrt_profile.argtypes = [ctypes.POINTER(ctypes.c_int64),
                                           ctypes.c_size_t]
    lib.axon_start_nrt_profile.restype = ctypes.c_int64
    lib.axon_stop_nrt_profile.argtypes = [ctypes.c_char_p]
    lib.axon_stop_nrt_profile.restype = ctypes.c_int64

    @contextlib.contextmanager
    def _hook(output_dir, device_ids):
        import jax
        jax.devices()
        if device_ids:
            ids = (ctypes.c_int64 * len(device_ids))(*device_ids)
            rc = lib.axon_start_nrt_profile(ids, len(device_ids))
        else:
            rc = lib.axon_start_nrt_profile(None, 0)
        if rc != 0:
            raise RuntimeError(f"axon_start_nrt_profile rc={rc}")
        try:
            yield
        finally:
            n = lib.axon_stop_nrt_profile(str(output_dir).encode())
            print(f"profile: {n} file(s) written to {output_dir}", file=sys.stderr)

    mod = types.ModuleType("antenv.axon_hooks")
    mod.get_axon_ntff_profile_hook = lambda: _hook
    mod.set_axon_ntff_profile_hook = lambda h: None
    sys.modules["antenv.axon_hooks"] = mod
    antenv.axon_hooks = mod


def _run(x, w_weight, w_bias, proj_weight, proj_bias, trace=False):
    from concourse.bass_utils import run_bass_kernel_spmd

    if trace:
        _install_ntff_hook()

    if "nc" not in _CACHE:
        _CACHE["nc"] = _build()
    nc = _CACHE["nc"]
    maps = _in_maps(x, w_weight, w_bias, proj_weight, proj_bias)
    res = run_bass_kernel_spmd(nc, maps, core_ids=list(range(NCORES)), trace=trace)
    out = np.zeros((B, T, C), np.float32)
    for c in range(NCORES):
        yc = res.results[c]["y"]  # [128, 8, 1024]
        b = c // 2
        h0 = (c % 2) * HPC
        for j in range(HPC):
            out[b, (h0 + j) * 128:(h0 + j + 1) * 128, :] = yc[:, j, :]
    return out, res.exec_time_ns


def kernel(x, w_weight, w_bias, proj_weight, proj_bias):
    out, _ = _run(x, w_weight, w_bias, proj_weight, proj_bias, trace=False)
    return out


def kernel_with_time(x, w_weight, w_bias, proj_weight, proj_bias):
    return _run(x, w_weight, w_bias, proj_weight, proj_bias, trace=True)


# revision 11
# speedup vs baseline: 1.1163x; 1.1163x over previous
"""Fused attention kernel for Trainium2, 8 NeuronCores.

Problem: B=4, T=2048, C=1024, nh=16, hs=64, fused QKV (chunk order k,q,v),
softmax attention, then (faithful reference bug) reshape (B,nh,T,hs)->(B,T,C)
directly before the output projection.

Key structural fact: with the buggy reshape, head h's attention output
occupies exactly rows [h*128, (h+1)*128) of the reshaped (T, C) matrix
(row tau = h*128 + t//16, col = (t%16)*64 + d). So everything after the
QKV projection is fully independent per (batch, head) pair; the output
projection needs no cross-head reduction.

Sharding: 8 cores = 4 batches x 2 head-groups (8 heads each). Each core
computes its batch's QKV slice and its 8 heads end-to-end. No collectives.
"""

import sys

import numpy as np

sys.path.insert(0, "/opt/trn_rl_repo")

import ml_dtypes  # noqa: E402

B, T, C = 4, 2048, 1024
NH, HS = 16, 64
NCORES = 8
HPC = 8  # heads per core

_CACHE = {}


def _build():
    from contextlib import ExitStack

    import concourse.bass as bass  # noqa: F401
    import concourse.mybir as mybir
    from concourse import bacc, tile

    F32 = mybir.dt.float32
    F32R = mybir.dt.float32r
    BF16 = mybir.dt.bfloat16
    ADD = mybir.AluOpType.add
    MULT = mybir.AluOpType.mult
    EXP = mybir.ActivationFunctionType.Exp

    nc = bacc.Bacc()
    xT = nc.dram_tensor("xT", [128, 8, 2048], F32R, kind="ExternalInput")
    wqkv = nc.dram_tensor("wqkv", [128, 8, 1536], F32R, kind="ExternalInput")
    bqk = nc.dram_tensor("bqk", [128, 8], F32, kind="ExternalInput")
    bv = nc.dram_tensor("bv", [128, 512], F32, kind="ExternalInput")
    wp = nc.dram_tensor("wp", [64, 16, 1024], BF16, kind="ExternalInput")
    pb = nc.dram_tensor("pb", [128, 1024], F32, kind="ExternalInput")
    vones = nc.dram_tensor("vones", [128, 16, 8], BF16, kind="ExternalInput")
    ones1 = nc.dram_tensor("ones1", [1, 64], F32R, kind="ExternalInput")
    y = nc.dram_tensor("y", [128, 8, 1024], F32, kind="ExternalOutput")

    with tile.TileContext(nc) as tc, ExitStack() as ctx:
        persist = ctx.enter_context(tc.tile_pool(name="persist", bufs=1))
        yps = ctx.enter_context(tc.tile_pool(name="ysb", bufs=2))

        scratch = persist.tile([128, 4], F32, tag="scratch")
        bqk_sb = persist.tile([128, 8], F32, tag="bqk")
        nc.sync.dma_start(bqk_sb, bqk[:])
        nc.vector.tensor_copy(scratch[:, 0:1], bqk_sb[:, 0:1])
        pb_sb = persist.tile([128, 1024], F32, tag="pb")
        nc.sync.dma_start(pb_sb, pb[:])
        nc.vector.tensor_copy(scratch[:, 1:2], pb_sb[:, 0:1])
        ones1_sb = persist.tile([1, 64], F32R, tag="ones1")
        nc.sync.dma_start(ones1_sb, ones1[:])
        vbuf = persist.tile([128, 16, HPC, 65], BF16, tag="vbuf")
        nc.sync.dma_start(vbuf[:, :, :, 64], vones[:])
        nc.vector.tensor_copy(scratch[:, 2:3], vbuf[:, 0, 0, 64:65])
        # K^T rows in tiles 0-3 (d on partitions, t free), Q^T rows in 4-7
        qk = [persist.tile([128, 2048], F32R, tag=f"qk{mt}", name=f"qk{mt}")
              for mt in range(8)]

        # ---------------- QKV phase ----------------
        with tc.tile_pool(name="qkvsb", bufs=1) as qsb, \
             tc.tile_pool(name="wstream", bufs=2) as wsp, \
             tc.tile_pool(name="qkvps", bufs=2, space="PSUM") as qps:
            bv_sb = qsb.tile([128, 512], F32, tag="bv")
            nc.sync.dma_start(bv_sb, bv[:])
            nc.vector.tensor_copy(scratch[:, 3:4], bv_sb[:, 0:1])
            xts = []
            for ct in range(8):
                t = qsb.tile([128, 2048], F32R, tag=f"xt{ct}")
                nc.sync.dma_start(t, xT[:, ct, :])
                xts.append(t)

            for mt in range(8):
                wt = wsp.tile([128, 8, 128], F32R, tag="wt")
                nc.sync.dma_start(wt, wqkv[:, :, mt * 128:(mt + 1) * 128])
                for icx in range(4):
                    ps = qps.tile([128, 512], F32, tag="qkvps")
                    for ct in range(8):
                        nc.tensor.matmul(
                            ps, wt[:, ct, :], xts[ct][:, icx * 512:(icx + 1) * 512],
                            start=(ct == 0), stop=(ct == 7))
                    nc.vector.tensor_tensor(
                        qk[mt][:, icx * 512:(icx + 1) * 512], ps,
                        bqk_sb[:, mt:mt + 1].to_broadcast((128, 512)), ADD)

            wv_sb = qsb.tile([128, 8, 512], F32R, tag="wv")
            nc.sync.dma_start(wv_sb, wqkv[:, :, 1024:1536])
            for tt in range(16):
                ps = qps.tile([128, 512], F32, tag="qkvps")
                for ct in range(8):
                    nc.tensor.matmul(
                        ps, xts[ct][:, tt * 128:(tt + 1) * 128], wv_sb[:, ct, :],
                        start=(ct == 0), stop=(ct == 7))
                nc.vector.tensor_tensor(
                    vbuf[:, tt, :, 0:64],
                    ps.rearrange("p (h d) -> p h d", d=64),
                    bv_sb.rearrange("p (h d) -> p h d", d=64), ADD)

        # ---------------- attention + projection ----------------
        with tc.tile_pool(name="attnsb", bufs=1) as asb, \
             tc.tile_pool(name="utp", bufs=4) as utp, \
             tc.tile_pool(name="otp", bufs=1) as otp, \
             tc.tile_pool(name="nrm", bufs=2) as nrm, \
             tc.tile_pool(name="dpool", bufs=2, space="DRAM") as dpool:
            wp_sb = asb.tile([64, 16, 1024], BF16, tag="wp")
            nc.sync.dma_start(wp_sb, wp[:])
            ots = [otp.tile([64, 2048], BF16, tag=f"ot{h}", name=f"ot{h}")
                   for h in range(HPC)]

            with tc.tile_pool(name="spool", bufs=2, space="PSUM") as spx, \
                 tc.tile_pool(name="opool", bufs=2, space="PSUM") as opx:
                for h in range(HPC):
                    mt = h // 2
                    pr = (h % 2) * 64
                    kh = qk[mt][pr:pr + 64, :]
                    qh = qk[4 + mt][pr:pr + 64, :]
                    ot = ots[h]
                    for ih in range(2):
                        optile = opx.tile([128, 1024], F32, tag="op")
                        for j in range(16):
                            sp = spx.tile([128, 1024], F32, tag="sp")
                            for q2 in range(2):
                                nc.tensor.matmul(
                                    sp[:, q2 * 512:(q2 + 1) * 512],
                                    kh[:, j * 128:(j + 1) * 128],
                                    qh[:, ih * 1024 + q2 * 512: ih * 1024 + (q2 + 1) * 512],
                                    start=True, stop=True)
                            ut = utp.tile([128, 1024], BF16, tag="ut")
                            nc.scalar.activation(ut, sp, EXP, scale=0.125)
                            for q2 in range(2):
                                nc.tensor.matmul(
                                    optile[0:65, q2 * 512:(q2 + 1) * 512],
                                    vbuf[:, j, h, :],
                                    ut[:, q2 * 512:(q2 + 1) * 512],
                                    start=(j == 0), stop=(j == 15))
                        # row 64 of optile[0:65] is the softmax denominator.
                        # Reshape it to [128, 8] via DRAM so reciprocal runs at
                        # 8 els/lane, then DMA-broadcast back to [64, 1024].
                        rs = nrm.tile([1, 1024], F32, tag="rs")
                        nc.vector.tensor_copy(rs, optile[64:65, :])
                        scr1 = dpool.tile([1024], F32, tag="scr1")
                        nc.sync.dma_start(scr1[None, :], rs)
                        rst = nrm.tile([128, 8], F32, tag="rst")
                        nc.sync.dma_start(rst, scr1.rearrange("(p f) -> p f", f=8))
                        nc.vector.reciprocal(rst, rst)
                        scr2 = dpool.tile([1024], F32, tag="scr2")
                        nc.sync.dma_start(scr2.rearrange("(p f) -> p f", f=8), rst)
                        bcsb = nrm.tile([64, 1024], F32, tag="bcsb")
                        nc.sync.dma_start(bcsb, scr2[None, :].to_broadcast((64, 1024)))
                        for q2 in range(2):
                            nc.vector.tensor_tensor(
                                ot[:, ih * 1024 + q2 * 512: ih * 1024 + (q2 + 1) * 512],
                                optile[0:64, q2 * 512:(q2 + 1) * 512],
                                bcsb[:, q2 * 512:(q2 + 1) * 512], MULT)

            # dense projection phase over all heads:
            # Y[tau, o] = sum_u sum_d OT[d, 16*tau+u] * wpT[u*64+d, o]
            with tc.tile_pool(name="ypool", bufs=2, space="PSUM") as ypx:
                for h in range(HPC):
                    yp = ypx.tile([128, 1024], F32, tag="yp")
                    otr = ots[h].rearrange("d (t u) -> d u t", u=16)
                    for u in range(16):
                        for q2 in range(2):
                            nc.tensor.matmul(
                                yp[:, q2 * 512:(q2 + 1) * 512], otr[:, u, :],
                                wp_sb[:, u, q2 * 512:(q2 + 1) * 512],
                                start=(u == 0), stop=(u == 15))
                    ysb = yps.tile([128, 1024], F32, tag="ysb")
                    nc.vector.tensor_tensor(ysb, yp, pb_sb, ADD)
                    nc.sync.dma_start(y[:, h, :], ysb)
    nc.compile()
    return nc


def _in_maps(x, w_weight, w_bias, proj_weight, proj_bias):
    x = np.ascontiguousarray(x, np.float32)
    w_weight = np.ascontiguousarray(w_weight, np.float32)
    w_bias = np.ascontiguousarray(w_bias, np.float32)
    proj_weight = np.ascontiguousarray(proj_weight, np.float32)
    proj_bias = np.ascontiguousarray(proj_bias, np.float32)

    wpT = np.ascontiguousarray(
        proj_weight.T.reshape(16, 64, 1024).transpose(1, 0, 2).astype(ml_dtypes.bfloat16))
    pbr = np.ascontiguousarray(np.tile(proj_bias[None], (128, 1)))
    vones = np.ones((128, 16, HPC), dtype=ml_dtypes.bfloat16)
    ones1 = np.ones((1, 64), np.float32)

    maps = []
    for c in range(NCORES):
        b = c // 2
        h0 = (c % 2) * HPC
        xTc = np.ascontiguousarray(
            x[b].T.reshape(8, 128, 2048).transpose(1, 0, 2))
        wk = w_weight[h0 * 64: h0 * 64 + 512]
        wq = w_weight[1024 + h0 * 64: 1024 + h0 * 64 + 512]
        wv = w_weight[2048 + h0 * 64: 2048 + h0 * 64 + 512]
        wqkvT = np.concatenate([wk.T, wq.T, wv.T], axis=1)  # [1024, 1536]
        wqkvT = np.ascontiguousarray(
            wqkvT.reshape(8, 128, 1536).transpose(1, 0, 2))
        bk = w_bias[h0 * 64: h0 * 64 + 512]
        bq = w_bias[1024 + h0 * 64: 1024 + h0 * 64 + 512]
        bvc = w_bias[2048 + h0 * 64: 2048 + h0 * 64 + 512]
        bqkc = np.ascontiguousarray(
            np.concatenate([bk.reshape(4, 128).T, bq.reshape(4, 128).T], axis=1))
        bvr = np.ascontiguousarray(np.tile(bvc[None], (128, 1)))
        maps.append({
            "xT": xTc, "wqkv": wqkvT, "bqk": bqkc, "bv": bvr,
            "wp": wpT, "pb": pbr, "vones": vones, "ones1": ones1,
        })
    return maps


def _install_ntff_hook():
    """Register the axon NTFF profiling hook (missing antenv.axon_hooks shim)."""
    import contextlib
    import ctypes
    import types

    if "antenv.axon_hooks" in sys.modules:
        return
    import antenv
    so_path = "/opt/axon/libaxon_pjrt.so"
    try:
        lib = ctypes.CDLL(so_path)
    except OSError:
        return
    if not hasattr(lib, "axon_start_nrt_profile"):
        return
    lib.axon_start_nrt_profile.argtypes = [ctypes.POINTER(ctypes.c_int64),
                                           ctypes.c_size_t]
    lib.axon_start_nrt_profile.restype = ctypes.c_int64
    lib.axon_stop_nrt_profile.argtypes = [ctypes.c_char_p]
    lib.axon_stop_nrt_profile.restype = ctypes.c_int64

    @contextlib.contextmanager
    def _hook(output_dir, device_ids):
        import jax
        jax.devices()
        if device_ids:
            ids = (ctypes.c_int64 * len(device_ids))(*device_ids)
            rc = lib.axon_start_nrt_profile(ids, len(device_ids))
        else:
            rc = lib.axon_start_nrt_profile(None, 0)
        if rc != 0:
            raise RuntimeError(f"axon_start_nrt_profile rc={rc}")
        try:
            yield
        finally:
            n = lib.axon_stop_nrt_profile(str(output_dir).encode())
            print(f"profile: {n} file(s) written to {output_dir}", file=sys.stderr)

    mod = types.ModuleType("antenv.axon_hooks")
    mod.get_axon_ntff_profile_hook = lambda: _hook
    mod.set_axon_ntff_profile_hook = lambda h: None
    sys.modules["antenv.axon_hooks"] = mod
    antenv.axon_hooks = mod


def _run(x, w_weight, w_bias, proj_weight, proj_bias, trace=False):
    from concourse.bass_utils import run_bass_kernel_spmd

    if trace:
        _install_ntff_hook()

    if "nc" not in _CACHE:
        _CACHE["nc"] = _build()
    nc = _CACHE["nc"]
    maps = _in_maps(x, w_weight, w_bias, proj_weight, proj_bias)
    res = run_bass_kernel_spmd(nc, maps, core_ids=list(range(NCORES)), trace=trace)
    out = np.zeros((B, T, C), np.float32)
    for c in range(NCORES):
        yc = res.results[c]["y"]  # [128, 8, 1024]
        b = c // 2
        h0 = (c % 2) * HPC
        for j in range(HPC):
            out[b, (h0 + j) * 128:(h0 + j + 1) * 128, :] = yc[:, j, :]
    return out, res.exec_time_ns


def kernel(x, w_weight, w_bias, proj_weight, proj_bias):
    out, _ = _run(x, w_weight, w_bias, proj_weight, proj_bias, trace=False)
    return out


def kernel_with_time(x, w_weight, w_bias, proj_weight, proj_bias):
    return _run(x, w_weight, w_bias, proj_weight, proj_bias, trace=True)
